# revision 17
# baseline (speedup 1.0000x reference)
"""BERT self-attention (B=4, S=2048, H=768, 12 heads) on 8 NeuronCores.

Sharding: core c handles batch b=c//2, query-half qh=c%2 (1024 q rows).
K/V are computed for the full sequence on each core (duplicated across the
2 cores of a batch) so no collectives are needed. Matmul operands are fp16
(PE runs fp16 at 1 cyc/row vs 4 for fp32; values here are O(1-40) so fp16
range is safe); accumulation stays fp32 in PSUM.

Pipeline per core (layouts chosen so the only transpose is the fp16 XBAR
DMA-transpose of the input):
  xT  [h,s]  <- DMA-transpose of x
  v   [s,h]  = xT.T @ Wv + bv, with a ones column per head (rowsum trick)
  per head-chunk hc: kT/qT chunk = W.T @ xT + b (q also *0.125), then
  attention for heads 2hc/2hc+1 interleaved so ACT exp overlaps the next
  chunk's projection matmuls. Head pair MMs alternate partition bases 0/64
  -> concurrent row-tiled execution on the PE array.
  scoresT[k,q] psum -> exp on ACT -> ctx^T accumulated via lhsT=v_aug
  (psum rows 0-63 = ctx^T, row 64 = softmax denominator).
  Normalization batched: rowsums gathered to [24,512], two reciprocals,
  per-group PE broadcast (selection matrix) + one DVE multiply in place.
  out = gelu(ctxU.T @ Wo + bo) (bias via K=1 ones matmul into psum).
"""

import sys

sys.path.insert(0, "/opt/trn_rl_repo")

import numpy as np

import concourse.bass as bass
import concourse.tile as tile
import concourse.mybir as mybir
from concourse.masks import make_identity

F16 = mybir.dt.float16
F32 = mybir.dt.float32
AF = mybir.ActivationFunctionType
ALU = mybir.AluOpType

S = 2048  # full sequence
SQ = 1024  # per-core query rows
H = 768  # hidden
NH = 12  # heads
DH = 64  # head dim
HC = H // 128  # 6 hidden chunks
SC = S // 128  # 16 seq chunks
QC = SQ // 128  # 8 query chunks
VW = DH + 1  # 65: V columns per head incl. ones column for rowsum
QN = SQ // 512  # 2 q-blocks per core
NG = NH * QN  # 24 (head, q-block) groups


def split_sync_waits(nc, cap=1):
    """Walrus in this container rejects instructions carrying more than ~1
    sync wait. Move excess waits onto same-engine NoOps inserted just before
    the instruction (same queue -> executed in order -> semantics kept)."""
    n = 0
    for b in nc.m.functions[0].blocks:
        out = []
        for inst in b.instructions:
            si = inst.sync_info
            waits = list(si.on_wait) if si is not None and si.on_wait else []
            if len(waits) > cap:
                extra, keep = waits[:-cap], waits[-cap:]
                for i in range(0, len(extra), cap):
                    nop = mybir.InstNoOp(
                        name=f"wsplit-{n}",
                        engine=inst.engine,
                        sync_info=mybir.SyncInfo(
                            on_wait=extra[i : i + cap], on_update=[]
                        ),
                    )
                    n += 1
                    out.append(nop)
                si.on_wait = keep
            out.append(inst)
        b.instructions[:] = out
    return n


def build_program():
    nc = bass.Bass()
    x = nc.declare_dram_parameter("x", [S, H], F16, isOutput=False)
    xq = nc.declare_dram_parameter("xq", [SQ, H], F16, isOutput=False)
    wq = nc.declare_dram_parameter("wq", [H, H], F16, isOutput=False)
    wk = nc.declare_dram_parameter("wk", [H, H], F16, isOutput=False)
    wv = nc.declare_dram_parameter("wv", [H, H], F16, isOutput=False)
    wo = nc.declare_dram_parameter("wo", [H, H], F16, isOutput=False)
    bqf = nc.declare_dram_parameter("bqf", [H], F32, isOutput=False)
    bkf = nc.declare_dram_parameter("bkf", [H], F32, isOutput=False)
    bv16 = nc.declare_dram_parameter("bv16", [H], F16, isOutput=False)
    bo16 = nc.declare_dram_parameter("bo16", [H], F16, isOutput=False)
    out = nc.declare_dram_parameter("out", [SQ, H], F32, isOutput=True)

    with tile.TileContext(nc) as tc:
        from contextlib import ExitStack

        with ExitStack() as ctx:
            consts = ctx.enter_context(tc.tile_pool(name="consts", bufs=1))
            wpool = ctx.enter_context(tc.tile_pool(name="wpool", bufs=1))
            big = ctx.enter_context(tc.tile_pool(name="big", bufs=1))
            copystage = ctx.enter_context(tc.tile_pool(name="copystage", bufs=4))
            outstage = ctx.enter_context(tc.tile_pool(name="outstage", bufs=2))
            pp_mm = ctx.enter_context(
                tc.tile_pool(name="pp_mm", bufs=2, space="PSUM")
            )
            pp_s = ctx.enter_context(tc.tile_pool(name="pp_s", bufs=2, space="PSUM"))
            pp_c = ctx.enter_context(tc.tile_pool(name="pp_c", bufs=2, space="PSUM"))

            # ---- constants ----
            ident = consts.tile([128, 128], F16, tag="ident")
            make_identity(nc, ident[:])
            ones16 = consts.tile([128, 512], F16, tag="ones16")
            nc.gpsimd.memset(ones16[:], 1.0)

            # ---- weights & biases to SBUF ----
            wq_sb = wpool.tile([128, HC, H], F16, tag="wq")
            wk_sb = wpool.tile([128, HC, H], F16, tag="wk")
            wv_sb = wpool.tile([128, HC, H], F16, tag="wv")
            wo_sb = wpool.tile([128, HC, H], F16, tag="wo")
            for w_sb, w in ((wv_sb, wv), (wk_sb, wk), (wq_sb, wq), (wo_sb, wo)):
                nc.gpsimd.dma_start(w_sb[:], w.rearrange("(c p) o -> p c o", p=128))
            bq_sb = wpool.tile([128, HC], F32, tag="bq")
            bk_sb = wpool.tile([128, HC], F32, tag="bk")
            nc.gpsimd.dma_start(bq_sb[:], bqf.rearrange("(c p) -> p c", p=128))
            nc.gpsimd.dma_start(bk_sb[:], bkf.rearrange("(c p) -> p c", p=128))
            bv_sb = wpool.tile([1, H], F16, tag="bv")
            bo_sb = wpool.tile([1, H], F16, tag="bo")
            nc.gpsimd.dma_start(bv_sb[:], bv16[None, :])
            nc.gpsimd.dma_start(bo_sb[:], bo16[None, :])

            # ---- x -> xT via XBAR DMA transpose (fp16), 512-row chunks ----
            xT = big.tile([128, HC, S], F16, tag="xT")
            xqT = big.tile([128, HC, SQ], F16, tag="xqT")
            for hc in range(HC):
                eng = nc.sync
                eng.dma_start_transpose(
                    xT[:, hc, :], x[:, hc * 128 : (hc + 1) * 128]
                )
                eng.dma_start_transpose(
                    xqT[:, hc, :], xq[:, hc * 128 : (hc + 1) * 128]
                )

            # ---- broadcast bias rows across 128 partitions (K=1 matmul) --
            bv_bc = wpool.tile([128, H], F32, tag="bv_bc")
            bo_bc = wpool.tile([128, H], F32, tag="bo_bc")
            for bc, bsb in ((bv_bc, bv_sb), (bo_bc, bo_sb)):
                for c0, cw in ((0, 512), (512, 256)):
                    ps = pp_mm.tile([128, 512], F32, tag="pp_mm")
                    nc.tensor.matmul(
                        ps[:, :cw],
                        ones16[0:1, 0:128],
                        bsb[:, c0 : c0 + cw],
                        start=True,
                        stop=True,
                    )
                    nc.vector.tensor_copy(bc[:, c0 : c0 + cw], ps[:, :cw])

            # ---- V (seq on partitions) with ones column per head ----
            v_sb = big.tile([128, SC, NH * VW], F16, tag="v")
            v_heads = v_sb[:].rearrange("p s (h c) -> p s h c", c=VW)
            nc.gpsimd.memset(v_heads[:, :, :, DH], 1.0)
            for sc in range(SC):
                for c0, cw in ((0, 512), (512, 256)):
                    ps = pp_mm.tile([128, 512], F32, tag="pp_mm")
                    for ic in range(HC):
                        nc.tensor.matmul(
                            ps[:, :cw],
                            xT[:, ic, sc * 128 : (sc + 1) * 128],
                            wv_sb[:, ic, c0 : c0 + cw],
                            start=(ic == 0),
                            stop=(ic == HC - 1),
                        )
                    h0 = c0 // DH
                    nhh = cw // DH
                    nc.vector.scalar_tensor_tensor(
                        v_heads[:, sc, h0 : h0 + nhh, 0:DH],
                        ps[:, :cw].rearrange("p (h c) -> p h c", c=DH),
                        1.0,
                        bv_bc[:, c0 : c0 + cw].rearrange("p (h c) -> p h c", c=DH),
                        ALU.mult,
                        ALU.add,
                    )

            kT = big.tile([128, HC, S], F16, tag="kT")
            qT = big.tile([128, HC, SQ], F16, tag="qT")
            ctxU = big.tile([128, HC, SQ], F16, tag="ctxU")
            # group g lives at padded row r(g): second half starts at
            # partition 32 (compute-engine partition bases must be 0/32/64/96)
            NR = 32 + NG // 2  # 44
            rows_sb = big.tile([NR, 512], F32, tag="rows")
            recip16 = big.tile([NR, 512], F16, tag="recip16")

            def grow(g):
                return g if g < NG // 2 else 32 + (g - NG // 2)

            def normalize_group(h, qn):
                """Broadcast 1/rowsum over 64 partitions via a selection-
                matrix matmul, then scale ctxU in place."""
                hb = (h % 2) * 64
                g = h * QN + qn
                r = grow(g)
                pb = pp_mm.tile([128, 512], F32, tag="pp_mm")
                nc.tensor.matmul(
                    pb[hb : hb + 64, :],
                    ident[0:NR, r : r + 1].to_broadcast([NR, 64]),
                    recip16[:],
                    start=True,
                    stop=True,
                )
                sl = ctxU[hb : hb + 64, h // 2, qn * 512 : (qn + 1) * 512]
                nc.vector.tensor_tensor(sl, sl, pb[hb : hb + 64, :], ALU.mult)

            # ---- per head-chunk: K/Q projection then paired attention ----
            for hc in range(HC):
                for w_sb, b_sb, dst, nsn, extra in (
                    (wk_sb, bk_sb, kT, S // 512, None),
                    (wq_sb, bq_sb, qT, QN, 0.125),
                ):
                    src = xT if dst is kT else xqT
                    for sn in range(nsn):
                        ps = pp_mm.tile([128, 512], F32, tag="pp_mm")
                        for ic in range(HC):
                            nc.tensor.matmul(
                                ps[:],
                                w_sb[:, ic, hc * 128 : (hc + 1) * 128],
                                src[:, ic, sn * 512 : (sn + 1) * 512],
                                start=(ic == 0),
                                stop=(ic == HC - 1),
                            )
                        if extra is None:
                            nc.vector.tensor_scalar_add(
                                dst[:, hc, sn * 512 : (sn + 1) * 512],
                                ps[:],
                                b_sb[:, hc : hc + 1],
                            )
                        else:
                            nc.vector.tensor_scalar(
                                dst[:, hc, sn * 512 : (sn + 1) * 512],
                                ps[:],
                                b_sb[:, hc : hc + 1],
                                extra,
                                ALU.add,
                                ALU.mult,
                            )

                hA, hB = 2 * hc, 2 * hc + 1
                for qn in range(QN):
                    pscA = pp_c.tile([VW, 512], F32, tag="pp_c")
                    pscB = pp_c.tile([VW, 512], F32, tag="pp_c")
                    for kc2 in range(SC // 2):
                        pssA = pp_s.tile([128, 1024], F32, tag="pp_s")
                        pssB = pp_s.tile([128, 1024], F32, tag="pp_s")
                        for j in range(2):
                            kc = kc2 * 2 + j
                            # A then B: bases 0 / 64 -> concurrent in array
                            for hb, pss in ((0, pssA), (64, pssB)):
                                nc.tensor.matmul(
                                    pss[:, j * 512 : (j + 1) * 512],
                                    kT[hb : hb + 64, hc, kc * 128 : (kc + 1) * 128],
                                    qT[hb : hb + 64, hc, qn * 512 : (qn + 1) * 512],
                                    start=True,
                                    stop=True,
                                )
                        etA = copystage.tile([128, 1024], F16, tag="et")
                        etB = copystage.tile([128, 1024], F16, tag="et")
                        nc.scalar.activation(etA[:], pssA[:], AF.Exp)
                        nc.scalar.activation(etB[:], pssB[:], AF.Exp)
                        for h, psc, et in ((hA, pscA, etA), (hB, pscB, etB)):
                            for j in range(2):
                                kc = kc2 * 2 + j
                                nc.tensor.matmul(
                                    psc[:],
                                    v_sb[:, kc, h * VW : (h + 1) * VW],
                                    et[:, j * 512 : (j + 1) * 512],
                                    start=(kc == 0),
                                    stop=(kc == SC - 1),
                                )
                    for h, psc in ((hA, pscA), (hB, pscB)):
                        hb = (h % 2) * 64
                        g = h * QN + qn
                        dst = ctxU[hb : hb + 64, h // 2, qn * 512 : (qn + 1) * 512]
                        if hb == 0:
                            nc.vector.tensor_copy(dst, psc[0:64, :])
                        else:
                            cst = copystage.tile([64, 512], F16, tag="cst")
                            nc.vector.tensor_copy(cst[:], psc[0:64, :])
                            nc.sync.dma_start(dst, cst[:])
                        rstage = copystage.tile([65, 512], F32, tag="rstage")
                        nc.vector.tensor_copy(rstage[64:65, :], psc[64:65, :])
                        r = grow(g)
                        nc.sync.dma_start(rows_sb[r : r + 1, :], rstage[64:65, :])

                # after half the heads are done, normalize that half so the
                # work overlaps the remaining attention chunks
                if hc == HC // 2 - 1 or hc == HC - 1:
                    first = hc == HC // 2 - 1
                    rlo = 0 if first else 32
                    rhi = rlo + NG // 2
                    glo = 0 if first else NG // 2
                    rec = copystage.tile([NR, 512], F32, tag="rec")
                    nc.vector.reciprocal(rec[rlo:rhi, :], rows_sb[rlo:rhi, :])
                    nc.vector.tensor_copy(recip16[rlo:rhi, :], rec[rlo:rhi, :])
                    for g in range(glo, glo + NG // 2):
                        normalize_group(g // QN, g % QN)

            # ---- output projection + bias + gelu ----
            out_t = out.rearrange("(n p) h -> n p h", p=128)
            for qc in range(QC):
                ost = outstage.tile([128, H], F32, tag="ost")
                for c0, cw in ((0, 512), (512, 256)):
                    ps = pp_mm.tile([128, 512], F32, tag="pp_mm")
                    for mc in range(HC):
                        nc.tensor.matmul(
                            ps[:, :cw],
                            ctxU[:, mc, qc * 128 : (qc + 1) * 128],
                            wo_sb[:, mc, c0 : c0 + cw],
                            start=(mc == 0),
                            stop=(mc == HC - 1),
                        )
                    pre = outstage.tile([128, 512], F32, tag="pre")
                    nc.vector.tensor_tensor(
                        pre[:, :cw], ps[:, :cw], bo_bc[:, c0 : c0 + cw], ALU.add
                    )
                    nc.scalar.activation(ost[:, c0 : c0 + cw], pre[:, :cw], AF.Gelu)
                nc.sync.dma_start(out_t[qc], ost[:])

    split_sync_waits(nc, cap=1)
    return nc


_NC_CACHE = None


def _get_nc():
    global _NC_CACHE
    if _NC_CACHE is None:
        _NC_CACHE = build_program()
    return _NC_CACHE


def _install_ntff_hook():
    """The image's antenv lacks axon_hooks; synthesize it so
    run_bass_kernel_spmd(trace=True) can reach the axon NTFF profiler."""
    import types

    if "antenv.axon_hooks" in sys.modules:
        return
    mod = types.ModuleType("antenv.axon_hooks")
    _h = [None]
    mod.set_axon_ntff_profile_hook = lambda h: _h.__setitem__(0, h)
    mod.get_axon_ntff_profile_hook = lambda: _h[0]
    sys.modules["antenv.axon_hooks"] = mod
    import antenv

    antenv.axon_hooks = mod
    from trn_agent_boot.trn_boot import _ntff_profile_via_ctypes

    hook = _ntff_profile_via_ctypes("/opt/axon/libaxon_pjrt.so")
    mod.set_axon_ntff_profile_hook(hook)


def kernel(
    hidden_states,
    attention_mask,
    Wq,
    bq,
    Wk,
    bk,
    Wv,
    bv,
    Wo,
    bo,
    _trace=False,
):
    from concourse.bass_utils import run_bass_kernel_spmd

    hs = np.asarray(hidden_states, dtype=np.float32)
    f16 = np.float16
    hs16 = hs.astype(f16)
    wq16 = np.asarray(Wq, dtype=np.float32).astype(f16)
    wk16 = np.asarray(Wk, dtype=np.float32).astype(f16)
    wv16 = np.asarray(Wv, dtype=np.float32).astype(f16)
    wo16 = np.asarray(Wo, dtype=np.float32).astype(f16)
    bqf = np.asarray(bq, dtype=np.float32)
    bkf = np.asarray(bk, dtype=np.float32)
    bv16v = np.asarray(bv, dtype=np.float32).astype(f16)
    bo16v = np.asarray(bo, dtype=np.float32).astype(f16)

    if _trace:
        _install_ntff_hook()
    nc = _get_nc()
    in_maps = []
    for c in range(8):
        b, qh = c // 2, c % 2
        in_maps.append(
            {
                "x": hs16[b],
                "xq": hs16[b, qh * SQ : (qh + 1) * SQ],
                "wq": wq16,
                "wk": wk16,
                "wv": wv16,
                "wo": wo16,
                "bqf": bqf,
                "bkf": bkf,
                "bv16": bv16v,
                "bo16": bo16v,
            }
        )
    res = run_bass_kernel_spmd(
        nc, in_maps, core_ids=list(range(8)), trace=_trace
    )
    if _trace:
        kernel.last_result = res
    B = hs.shape[0]
    full = np.empty((B, S, H), dtype=np.float32)
    for c in range(8):
        b, qh = c // 2, c % 2
        full[b, qh * SQ : (qh + 1) * SQ] = res.results[c]["out"]
    return full


# revision 19
# speedup vs baseline: 1.0341x; 1.0341x over previous
"""BERT self-attention (B=4, S=2048, H=768, 12 heads) on 8 NeuronCores.

Sharding: core c handles batch b=c//2, query-half qh=c%2 (1024 q rows).
K/V are computed for the full sequence on each core (duplicated across the
2 cores of a batch) so no collectives are needed. Matmul operands are fp16
(PE runs fp16 at 1 cyc/row vs 4 for fp32; values here are O(1-40) so fp16
range is safe); accumulation stays fp32 in PSUM.

Pipeline per core (layouts chosen so the only transpose is the fp16 XBAR
DMA-transpose of the input):
  xT  [h,s]  <- DMA-transpose of x
  v   [s,h]  = xT.T @ Wv + bv, with a ones column per head (rowsum trick)
  per head-chunk hc: kT/qT chunk = W.T @ xT + b (q also *0.125), then
  attention for heads 2hc/2hc+1 interleaved so ACT exp overlaps the next
  chunk's projection matmuls. Head pair MMs alternate partition bases 0/64
  -> concurrent row-tiled execution on the PE array.
  scoresT[k,q] psum -> exp on ACT -> ctx^T accumulated via lhsT=v_aug
  (psum rows 0-63 = ctx^T, row 64 = softmax denominator).
  Normalization batched: rowsums gathered to [24,512], two reciprocals,
  per-group PE broadcast (selection matrix) + one DVE multiply in place.
  out = gelu(ctxU.T @ Wo + bo) (bias via K=1 ones matmul into psum).
"""

import sys

sys.path.insert(0, "/opt/trn_rl_repo")

import numpy as np

import concourse.bass as bass
import concourse.tile as tile
import concourse.mybir as mybir
from concourse.masks import make_identity

F16 = mybir.dt.float16
F32 = mybir.dt.float32
AF = mybir.ActivationFunctionType
ALU = mybir.AluOpType

S = 2048  # full sequence
SQ = 1024  # per-core query rows
H = 768  # hidden
NH = 12  # heads
DH = 64  # head dim
HC = H // 128  # 6 hidden chunks
SC = S // 128  # 16 seq chunks
QC = SQ // 128  # 8 query chunks
VW = DH + 1  # 65: V columns per head incl. ones column for rowsum
QN = SQ // 512  # 2 q-blocks per core
NG = NH * QN  # 24 (head, q-block) groups


def split_sync_waits(nc, cap=1):
    """Walrus in this container rejects instructions carrying more than ~1
    sync wait. Move excess waits onto same-engine NoOps inserted just before
    the instruction (same queue -> executed in order -> semantics kept)."""
    n = 0
    for b in nc.m.functions[0].blocks:
        out = []
        for inst in b.instructions:
            si = inst.sync_info
            waits = list(si.on_wait) if si is not None and si.on_wait else []
            if len(waits) > cap:
                extra, keep = waits[:-cap], waits[-cap:]
                for i in range(0, len(extra), cap):
                    nop = mybir.InstNoOp(
                        name=f"wsplit-{n}",
                        engine=inst.engine,
                        sync_info=mybir.SyncInfo(
                            on_wait=extra[i : i + cap], on_update=[]
                        ),
                    )
                    n += 1
                    out.append(nop)
                si.on_wait = keep
            out.append(inst)
        b.instructions[:] = out
    return n


def build_program():
    nc = bass.Bass()
    x = nc.declare_dram_parameter("x", [S, H], F16, isOutput=False)
    wq = nc.declare_dram_parameter("wq", [H, H], F16, isOutput=False)
    wk = nc.declare_dram_parameter("wk", [H, H], F16, isOutput=False)
    wv = nc.declare_dram_parameter("wv", [H, H], F16, isOutput=False)
    wo = nc.declare_dram_parameter("wo", [H, H], F16, isOutput=False)
    bqf = nc.declare_dram_parameter("bqf", [H], F32, isOutput=False)
    bkf = nc.declare_dram_parameter("bkf", [H], F32, isOutput=False)
    bv16 = nc.declare_dram_parameter("bv16", [H], F16, isOutput=False)
    bo16 = nc.declare_dram_parameter("bo16", [H], F16, isOutput=False)
    out = nc.declare_dram_parameter("out", [SQ, H], F32, isOutput=True)

    with tile.TileContext(nc) as tc:
        from contextlib import ExitStack

        with ExitStack() as ctx:
            consts = ctx.enter_context(tc.tile_pool(name="consts", bufs=1))
            wpool = ctx.enter_context(tc.tile_pool(name="wpool", bufs=1))
            big = ctx.enter_context(tc.tile_pool(name="big", bufs=1))
            copystage = ctx.enter_context(tc.tile_pool(name="copystage", bufs=4))
            etpool = ctx.enter_context(tc.tile_pool(name="etpool", bufs=6))
            outstage = ctx.enter_context(tc.tile_pool(name="outstage", bufs=2))
            pp_mm = ctx.enter_context(
                tc.tile_pool(name="pp_mm", bufs=2, space="PSUM")
            )
            pp_s = ctx.enter_context(tc.tile_pool(name="pp_s", bufs=2, space="PSUM"))
            pp_c = ctx.enter_context(tc.tile_pool(name="pp_c", bufs=2, space="PSUM"))

            # ---- constants ----
            ident = consts.tile([128, 128], F16, tag="ident")
            make_identity(nc, ident[:])
            ones16 = consts.tile([128, 512], F16, tag="ones16")
            nc.gpsimd.memset(ones16[:], 1.0)

            # ---- weights & biases to SBUF ----
            wq_sb = wpool.tile([128, HC, H], F16, tag="wq")
            wk_sb = wpool.tile([128, HC, H], F16, tag="wk")
            wv_sb = wpool.tile([128, HC, H], F16, tag="wv")
            wo_sb = wpool.tile([128, HC, H], F16, tag="wo")
            for w_sb, w in ((wv_sb, wv), (wk_sb, wk), (wq_sb, wq), (wo_sb, wo)):
                nc.gpsimd.dma_start(w_sb[:], w.rearrange("(c p) o -> p c o", p=128))
            bq_sb = wpool.tile([128, HC], F32, tag="bq")
            bk_sb = wpool.tile([128, HC], F32, tag="bk")
            nc.gpsimd.dma_start(bq_sb[:], bqf.rearrange("(c p) -> p c", p=128))
            nc.gpsimd.dma_start(bk_sb[:], bkf.rearrange("(c p) -> p c", p=128))
            bv_sb = wpool.tile([1, H], F16, tag="bv")
            bo_sb = wpool.tile([1, H], F16, tag="bo")
            nc.gpsimd.dma_start(bv_sb[:], bv16[None, :])
            nc.gpsimd.dma_start(bo_sb[:], bo16[None, :])

            # ---- x -> xT via XBAR DMA transpose (fp16), 512-row chunks ----
            # queries are rows 0:SQ of x (host rotates the sequence so this
            # core's query half comes first; softmax over k is permutation-
            # invariant so K/V order doesn't matter)
            xT = big.tile([128, HC, S], F16, tag="xT")
            for hc in range(HC):
                nc.sync.dma_start_transpose(
                    xT[:, hc, :], x[:, hc * 128 : (hc + 1) * 128]
                )

            # ---- broadcast bias rows across 128 partitions (K=1 matmul) --
            bv_bc = wpool.tile([128, H], F32, tag="bv_bc")
            bo_bc = wpool.tile([128, H], F32, tag="bo_bc")
            for bc, bsb in ((bv_bc, bv_sb), (bo_bc, bo_sb)):
                for c0, cw in ((0, 512), (512, 256)):
                    ps = pp_mm.tile([128, 512], F32, tag="pp_mm")
                    nc.tensor.matmul(
                        ps[:, :cw],
                        ones16[0:1, 0:128],
                        bsb[:, c0 : c0 + cw],
                        start=True,
                        stop=True,
                    )
                    nc.vector.tensor_copy(bc[:, c0 : c0 + cw], ps[:, :cw])

            # ---- V (seq on partitions) with ones column per head ----
            v_sb = big.tile([128, SC, NH * VW], F16, tag="v")
            v_heads = v_sb[:].rearrange("p s (h c) -> p s h c", c=VW)
            nc.gpsimd.memset(v_heads[:, :, :, DH], 1.0)
            for sc in range(SC):
                for c0, cw in ((0, 512), (512, 256)):
                    ps = pp_mm.tile([128, 512], F32, tag="pp_mm")
                    for ic in range(HC):
                        nc.tensor.matmul(
                            ps[:, :cw],
                            xT[:, ic, sc * 128 : (sc + 1) * 128],
                            wv_sb[:, ic, c0 : c0 + cw],
                            start=(ic == 0),
                            stop=(ic == HC - 1),
                        )
                    h0 = c0 // DH
                    nhh = cw // DH
                    nc.vector.scalar_tensor_tensor(
                        v_heads[:, sc, h0 : h0 + nhh, 0:DH],
                        ps[:, :cw].rearrange("p (h c) -> p h c", c=DH),
                        1.0,
                        bv_bc[:, c0 : c0 + cw].rearrange("p (h c) -> p h c", c=DH),
                        ALU.mult,
                        ALU.add,
                    )

            kT = big.tile([128, HC, S], F16, tag="kT")
            qT = big.tile([128, HC, SQ], F16, tag="qT")
            ctxU = big.tile([128, HC, SQ], F16, tag="ctxU")
            # group g lives at padded row r(g): second half starts at
            # partition 32 (compute-engine partition bases must be 0/32/64/96)
            NR = 32 + NG // 2  # 44
            rows_sb = big.tile([NR, 512], F32, tag="rows")
            recip16 = big.tile([NR, 512], F16, tag="recip16")
            nc.gpsimd.memset(recip16[:], 0.0)

            def grow(g):
                return g if g < NG // 2 else 32 + (g - NG // 2)

            def normalize_group(h, qn):
                """Broadcast 1/rowsum over 64 partitions via a selection-
                matrix matmul, then scale ctxU in place."""
                hb = (h % 2) * 64
                g = h * QN + qn
                r = grow(g)
                pb = pp_mm.tile([128, 512], F32, tag="pp_mm")
                nc.tensor.matmul(
                    pb[hb : hb + 64, :],
                    ident[0:NR, r : r + 1].to_broadcast([NR, 64]),
                    recip16[:],
                    start=True,
                    stop=True,
                )
                sl = ctxU[hb : hb + 64, h // 2, qn * 512 : (qn + 1) * 512]
                nc.vector.tensor_tensor(sl, sl, pb[hb : hb + 64, :], ALU.mult)

            # ---- per head-chunk: K/Q projection then paired attention ----
            for hc in range(HC):
                for w_sb, b_sb, dst, nsn, extra in (
                    (wk_sb, bk_sb, kT, S // 512, None),
                    (wq_sb, bq_sb, qT, QN, 0.125),
                ):
                    src = xT
                    for sn in range(nsn):
                        ps = pp_mm.tile([128, 512], F32, tag="pp_mm")
                        for ic in range(HC):
                            nc.tensor.matmul(
                                ps[:],
                                w_sb[:, ic, hc * 128 : (hc + 1) * 128],
                                src[:, ic, sn * 512 : (sn + 1) * 512],
                                start=(ic == 0),
                                stop=(ic == HC - 1),
                            )
                        if extra is None:
                            nc.vector.tensor_scalar_add(
                                dst[:, hc, sn * 512 : (sn + 1) * 512],
                                ps[:],
                                b_sb[:, hc : hc + 1],
                            )
                        else:
                            nc.vector.tensor_scalar(
                                dst[:, hc, sn * 512 : (sn + 1) * 512],
                                ps[:],
                                b_sb[:, hc : hc + 1],
                                extra,
                                ALU.add,
                                ALU.mult,
                            )

                hA, hB = 2 * hc, 2 * hc + 1
                for qn in range(QN):
                    pscA = pp_c.tile([VW, 512], F32, tag="pp_c")
                    pscB = pp_c.tile([VW, 512], F32, tag="pp_c")
                    for kc2 in range(SC // 2):
                        pssA = pp_s.tile([128, 1024], F32, tag="pp_s")
                        pssB = pp_s.tile([128, 1024], F32, tag="pp_s")
                        for j in range(2):
                            kc = kc2 * 2 + j
                            # A then B: bases 0 / 64 -> concurrent in array
                            for hb, pss in ((0, pssA), (64, pssB)):
                                nc.tensor.matmul(
                                    pss[:, j * 512 : (j + 1) * 512],
                                    kT[hb : hb + 64, hc, kc * 128 : (kc + 1) * 128],
                                    qT[hb : hb + 64, hc, qn * 512 : (qn + 1) * 512],
                                    start=True,
                                    stop=True,
                                )
                        etA = etpool.tile([128, 1024], F16, tag="et")
                        etB = etpool.tile([128, 1024], F16, tag="et")
                        nc.scalar.activation(etA[:], pssA[:], AF.Exp)
                        nc.scalar.activation(etB[:], pssB[:], AF.Exp)
                        for h, psc, et in ((hA, pscA, etA), (hB, pscB, etB)):
                            for j in range(2):
                                kc = kc2 * 2 + j
                                nc.tensor.matmul(
                                    psc[:],
                                    v_sb[:, kc, h * VW : (h + 1) * VW],
                                    et[:, j * 512 : (j + 1) * 512],
                                    start=(kc == 0),
                                    stop=(kc == SC - 1),
                                )
                    for h, psc in ((hA, pscA), (hB, pscB)):
                        hb = (h % 2) * 64
                        g = h * QN + qn
                        dst = ctxU[hb : hb + 64, h // 2, qn * 512 : (qn + 1) * 512]
                        if hb == 0:
                            nc.vector.tensor_copy(dst, psc[0:64, :])
                        else:
                            cst = copystage.tile([64, 512], F16, tag="cst")
                            nc.vector.tensor_copy(cst[:], psc[0:64, :])
                            nc.sync.dma_start(dst, cst[:])
                        rstage = copystage.tile([65, 512], F32, tag="rstage")
                        nc.vector.tensor_copy(rstage[64:65, :], psc[64:65, :])
                        r = grow(g)
                        nc.sync.dma_start(rows_sb[r : r + 1, :], rstage[64:65, :])

                # after half the heads are done, normalize that half so the
                # work overlaps the remaining attention chunks
                if hc == HC // 2 - 1 or hc == HC - 1:
                    first = hc == HC // 2 - 1
                    rlo = 0 if first else 32
                    rhi = rlo + NG // 2
                    glo = 0 if first else NG // 2
                    rec = copystage.tile([NR, 512], F32, tag="rec")
                    nc.vector.reciprocal(rec[rlo:rhi, :], rows_sb[rlo:rhi, :])
                    nc.vector.tensor_copy(recip16[rlo:rhi, :], rec[rlo:rhi, :])
                    for g in range(glo, glo + NG // 2):
                        normalize_group(g // QN, g % QN)

            # ---- output projection + bias + gelu ----
            out_t = out.rearrange("(n p) h -> n p h", p=128)
            for qc in range(QC):
                ost = outstage.tile([128, H], F32, tag="ost")
                for c0, cw in ((0, 512), (512, 256)):
                    ps = pp_mm.tile([128, 512], F32, tag="pp_mm")
                    for mc in range(HC):
                        nc.tensor.matmul(
                            ps[:, :cw],
                            ctxU[:, mc, qc * 128 : (qc + 1) * 128],
                            wo_sb[:, mc, c0 : c0 + cw],
                            start=(mc == 0),
                            stop=(mc == HC - 1),
                        )
                    pre = outstage.tile([128, 512], F32, tag="pre")
                    nc.vector.tensor_tensor(
                        pre[:, :cw], ps[:, :cw], bo_bc[:, c0 : c0 + cw], ALU.add
                    )
                    nc.scalar.activation(ost[:, c0 : c0 + cw], pre[:, :cw], AF.Gelu)
                nc.sync.dma_start(out_t[qc], ost[:])

    split_sync_waits(nc, cap=1)
    return nc


_NC_CACHE = None


def _get_nc():
    global _NC_CACHE
    if _NC_CACHE is None:
        _NC_CACHE = build_program()
    return _NC_CACHE


def _install_ntff_hook():
    """The image's antenv lacks axon_hooks; synthesize it so
    run_bass_kernel_spmd(trace=True) can reach the axon NTFF profiler."""
    import types

    if "antenv.axon_hooks" in sys.modules:
        return
    mod = types.ModuleType("antenv.axon_hooks")
    _h = [None]
    mod.set_axon_ntff_profile_hook = lambda h: _h.__setitem__(0, h)
    mod.get_axon_ntff_profile_hook = lambda: _h[0]
    sys.modules["antenv.axon_hooks"] = mod
    import antenv

    antenv.axon_hooks = mod
    from trn_agent_boot.trn_boot import _ntff_profile_via_ctypes

    hook = _ntff_profile_via_ctypes("/opt/axon/libaxon_pjrt.so")
    mod.set_axon_ntff_profile_hook(hook)


def kernel(
    hidden_states,
    attention_mask,
    Wq,
    bq,
    Wk,
    bk,
    Wv,
    bv,
    Wo,
    bo,
    _trace=False,
):
    from concourse.bass_utils import run_bass_kernel_spmd

    hs = np.asarray(hidden_states, dtype=np.float32)
    f16 = np.float16
    hs16 = hs.astype(f16)
    wq16 = np.asarray(Wq, dtype=np.float32).astype(f16)
    wk16 = np.asarray(Wk, dtype=np.float32).astype(f16)
    wv16 = np.asarray(Wv, dtype=np.float32).astype(f16)
    wo16 = np.asarray(Wo, dtype=np.float32).astype(f16)
    bqf = np.asarray(bq, dtype=np.float32)
    bkf = np.asarray(bk, dtype=np.float32)
    bv16v = np.asarray(bv, dtype=np.float32).astype(f16)
    bo16v = np.asarray(bo, dtype=np.float32).astype(f16)

    if _trace:
        _install_ntff_hook()
    nc = _get_nc()
    in_maps = []
    for c in range(8):
        b, qh = c // 2, c % 2
        xc = hs16[b] if qh == 0 else np.concatenate(
            [hs16[b, SQ:], hs16[b, :SQ]], axis=0
        )
        in_maps.append(
            {
                "x": xc,
                "wq": wq16,
                "wk": wk16,
                "wv": wv16,
                "wo": wo16,
                "bqf": bqf,
                "bkf": bkf,
                "bv16": bv16v,
                "bo16": bo16v,
            }
        )
    res = run_bass_kernel_spmd(
        nc, in_maps, core_ids=list(range(8)), trace=_trace
    )
    if _trace:
        kernel.last_result = res
    B = hs.shape[0]
    full = np.empty((B, S, H), dtype=np.float32)
    for c in range(8):
        b, qh = c // 2, c % 2
        full[b, qh * SQ : (qh + 1) * SQ] = res.results[c]["out"]
    return full


# revision 21
# speedup vs baseline: 1.1238x; 1.0867x over previous
"""BERT self-attention (B=4, S=2048, H=768, 12 heads) on 8 NeuronCores.

Sharding: core c handles batch b=c//2, query-half qh=c%2 (1024 q rows).
K/V are computed for the full sequence on each core (duplicated across the
2 cores of a batch) so no collectives are needed. Matmul operands are fp16
(PE runs fp16 at 1 cyc/row vs 4 for fp32; values here are O(1-40) so fp16
range is safe); accumulation stays fp32 in PSUM.

Pipeline per core (layouts chosen so the only transpose is the fp16 XBAR
DMA-transpose of the input):
  xT  [h,s]  <- DMA-transpose of x
  v   [s,h]  = xT.T @ Wv + bv, with a ones column per head (rowsum trick)
  per head-chunk hc: kT/qT chunk = W.T @ xT + b (q also *0.125), then
  attention for heads 2hc/2hc+1 interleaved so ACT exp overlaps the next
  chunk's projection matmuls. Head pair MMs alternate partition bases 0/64
  -> concurrent row-tiled execution on the PE array.
  scoresT[k,q] psum -> exp on ACT -> ctx^T accumulated via lhsT=v_aug
  (psum rows 0-63 = ctx^T, row 64 = softmax denominator).
  Normalization batched: rowsums gathered to [24,512], two reciprocals,
  per-group PE broadcast (selection matrix) + one DVE multiply in place.
  out = gelu(ctxU.T @ Wo + bo) (bias via K=1 ones matmul into psum).
"""

import sys

sys.path.insert(0, "/opt/trn_rl_repo")

import numpy as np

import concourse.bass as bass
import concourse.tile as tile
import concourse.mybir as mybir
from concourse.masks import make_identity

F16 = mybir.dt.float16
F32 = mybir.dt.float32
AF = mybir.ActivationFunctionType
ALU = mybir.AluOpType

S = 2048  # full sequence
SQ = 1024  # per-core query rows
H = 768  # hidden
NH = 12  # heads
DH = 64  # head dim
HC = H // 128  # 6 hidden chunks
SC = S // 128  # 16 seq chunks
QC = SQ // 128  # 8 query chunks
VW = DH + 1  # 65: V columns per head incl. ones column for rowsum
QN = SQ // 512  # 2 q-blocks per core
NG = NH * QN  # 24 (head, q-block) groups


def split_sync_waits(nc, cap=1):
    """Walrus in this container rejects instructions carrying more than ~1
    sync wait. Move excess waits onto same-engine NoOps inserted just before
    the instruction (same queue -> executed in order -> semantics kept)."""
    n = 0
    for b in nc.m.functions[0].blocks:
        out = []
        for inst in b.instructions:
            si = inst.sync_info
            waits = list(si.on_wait) if si is not None and si.on_wait else []
            if len(waits) > cap:
                extra, keep = waits[:-cap], waits[-cap:]
                for i in range(0, len(extra), cap):
                    nop = mybir.InstNoOp(
                        name=f"wsplit-{n}",
                        engine=inst.engine,
                        sync_info=mybir.SyncInfo(
                            on_wait=extra[i : i + cap], on_update=[]
                        ),
                    )
                    n += 1
                    out.append(nop)
                si.on_wait = keep
            out.append(inst)
        b.instructions[:] = out
    return n


def build_program():
    nc = bass.Bass()
    x = nc.declare_dram_parameter("x", [S, H], F16, isOutput=False)
    wq = nc.declare_dram_parameter("wq", [H, H], F16, isOutput=False)
    wk = nc.declare_dram_parameter("wk", [H, H], F16, isOutput=False)
    wv = nc.declare_dram_parameter("wv", [H, H], F16, isOutput=False)
    wo = nc.declare_dram_parameter("wo", [H, H], F16, isOutput=False)
    bqf = nc.declare_dram_parameter("bqf", [H], F32, isOutput=False)
    bkf = nc.declare_dram_parameter("bkf", [H], F32, isOutput=False)
    bv16 = nc.declare_dram_parameter("bv16", [H], F16, isOutput=False)
    bo16 = nc.declare_dram_parameter("bo16", [H], F16, isOutput=False)
    out = nc.declare_dram_parameter("out", [SQ, H], F32, isOutput=True)

    with tile.TileContext(nc) as tc:
        from contextlib import ExitStack

        with ExitStack() as ctx:
            consts = ctx.enter_context(tc.tile_pool(name="consts", bufs=1))
            wpool = ctx.enter_context(tc.tile_pool(name="wpool", bufs=1))
            big = ctx.enter_context(tc.tile_pool(name="big", bufs=1))
            copystage = ctx.enter_context(tc.tile_pool(name="copystage", bufs=4))
            etpool = ctx.enter_context(tc.tile_pool(name="etpool", bufs=6))
            outstage = ctx.enter_context(tc.tile_pool(name="outstage", bufs=2))
            pp_mm = ctx.enter_context(
                tc.tile_pool(name="pp_mm", bufs=2, space="PSUM")
            )

            # ---- constants ----
            ident = consts.tile([128, 128], F16, tag="ident")
            make_identity(nc, ident[:])
            ones16 = consts.tile([128, 512], F16, tag="ones16")
            nc.gpsimd.memset(ones16[:], 1.0)

            # ---- weights & biases to SBUF ----
            wq_sb = wpool.tile([128, HC, H], F16, tag="wq")
            wk_sb = wpool.tile([128, HC, H], F16, tag="wk")
            wv_sb = wpool.tile([128, HC, H], F16, tag="wv")
            wo_sb = wpool.tile([128, HC, H], F16, tag="wo")
            for w_sb, w in ((wv_sb, wv), (wk_sb, wk), (wq_sb, wq)):
                nc.scalar.dma_start(w_sb[:], w.rearrange("(c p) o -> p c o", p=128))
            nc.gpsimd.dma_start(wo_sb[:], wo.rearrange("(c p) o -> p c o", p=128))
            bq_sb = wpool.tile([128, HC], F32, tag="bq")
            bk_sb = wpool.tile([128, HC], F32, tag="bk")
            nc.gpsimd.dma_start(bq_sb[:], bqf.rearrange("(c p) -> p c", p=128))
            nc.gpsimd.dma_start(bk_sb[:], bkf.rearrange("(c p) -> p c", p=128))
            bv_sb = wpool.tile([1, H], F16, tag="bv")
            bo_sb = wpool.tile([1, H], F16, tag="bo")
            nc.gpsimd.dma_start(bv_sb[:], bv16[None, :])
            nc.gpsimd.dma_start(bo_sb[:], bo16[None, :])

            # ---- x -> xT via XBAR DMA transpose (fp16), 512-row chunks ----
            # queries are rows 0:SQ of x (host rotates the sequence so this
            # core's query half comes first; softmax over k is permutation-
            # invariant so K/V order doesn't matter). Transpose on the (idle)
            # PE: the XBAR DMA-transpose path only sustains ~60 GB/s.
            xT = big.tile([128, HC, S], F16, tag="xT")
            x_t = x.rearrange("(n p) h -> n p h", p=128)
            with tc.tile_pool(name="xstage", bufs=4) as xstage, tc.tile_pool(
                name="pp_t", bufs=2, space="PSUM"
            ) as pp_t:
                for sc in range(SC):
                    xt = xstage.tile([128, H], F16, tag="xt")
                    nc.sync.dma_start(xt[:], x_t[sc])
                    for hc in range(HC):
                        pst = pp_t.tile([128, 128], F16, tag="pp_t")
                        nc.tensor.transpose(
                            pst[:], xt[:, hc * 128 : (hc + 1) * 128], ident[:]
                        )
                        nc.vector.tensor_copy(
                            xT[:, hc, sc * 128 : (sc + 1) * 128], pst[:]
                        )

            # ---- broadcast bias rows across 128 partitions (K=1 matmul) --
            bv_bc = wpool.tile([128, H], F32, tag="bv_bc")
            bo_bc = wpool.tile([128, H], F32, tag="bo_bc")
            for bc, bsb in ((bv_bc, bv_sb), (bo_bc, bo_sb)):
                for c0, cw in ((0, 512), (512, 256)):
                    ps = pp_mm.tile([128, 512], F32, tag="pp_mm")
                    nc.tensor.matmul(
                        ps[:, :cw],
                        ones16[0:1, 0:128],
                        bsb[:, c0 : c0 + cw],
                        start=True,
                        stop=True,
                    )
                    nc.vector.tensor_copy(bc[:, c0 : c0 + cw], ps[:, :cw])

            # ---- V (seq on partitions) with ones column per head ----
            v_sb = big.tile([128, SC, NH * VW], F16, tag="v")
            v_heads = v_sb[:].rearrange("p s (h c) -> p s h c", c=VW)
            nc.gpsimd.memset(v_heads[:, :, :, DH], 1.0)
            for sc in range(SC):
                for c0, cw in ((0, 512), (512, 256)):
                    ps = pp_mm.tile([128, 512], F32, tag="pp_mm")
                    for ic in range(HC):
                        nc.tensor.matmul(
                            ps[:, :cw],
                            xT[:, ic, sc * 128 : (sc + 1) * 128],
                            wv_sb[:, ic, c0 : c0 + cw],
                            start=(ic == 0),
                            stop=(ic == HC - 1),
                        )
                    h0 = c0 // DH
                    nhh = cw // DH
                    nc.vector.scalar_tensor_tensor(
                        v_heads[:, sc, h0 : h0 + nhh, 0:DH],
                        ps[:, :cw].rearrange("p (h c) -> p h c", c=DH),
                        1.0,
                        bv_bc[:, c0 : c0 + cw].rearrange("p (h c) -> p h c", c=DH),
                        ALU.mult,
                        ALU.add,
                    )

            pp_s = ctx.enter_context(tc.tile_pool(name="pp_s", bufs=2, space="PSUM"))
            pp_c = ctx.enter_context(tc.tile_pool(name="pp_c", bufs=2, space="PSUM"))
            kT = big.tile([128, HC, S], F16, tag="kT")
            qT = big.tile([128, HC, SQ], F16, tag="qT")
            ctxU = big.tile([128, HC, SQ], F16, tag="ctxU")
            # group g lives at padded row r(g): second half starts at
            # partition 32 (compute-engine partition bases must be 0/32/64/96)
            NR = 32 + NG // 2  # 44
            rows_sb = big.tile([NR, 512], F32, tag="rows")
            recip16 = big.tile([NR, 512], F16, tag="recip16")
            nc.gpsimd.memset(recip16[:], 0.0)

            def grow(g):
                return g if g < NG // 2 else 32 + (g - NG // 2)

            def normalize_group(h, qn):
                """Broadcast 1/rowsum over 64 partitions via a selection-
                matrix matmul, then scale ctxU in place."""
                hb = (h % 2) * 64
                g = h * QN + qn
                r = grow(g)
                pb = pp_mm.tile([128, 512], F32, tag="pp_mm")
                nc.tensor.matmul(
                    pb[hb : hb + 64, :],
                    ident[0:NR, r : r + 1].to_broadcast([NR, 64]),
                    recip16[:],
                    start=True,
                    stop=True,
                )
                sl = ctxU[hb : hb + 64, h // 2, qn * 512 : (qn + 1) * 512]
                nc.vector.tensor_tensor(sl, sl, pb[hb : hb + 64, :], ALU.mult)

            # ---- per head-chunk: K/Q projection then paired attention ----
            for hc in range(HC):
                for w_sb, b_sb, dst, nsn, extra in (
                    (wk_sb, bk_sb, kT, S // 512, None),
                    (wq_sb, bq_sb, qT, QN, 0.125),
                ):
                    src = xT
                    for sn in range(nsn):
                        ps = pp_mm.tile([128, 512], F32, tag="pp_mm")
                        for ic in range(HC):
                            nc.tensor.matmul(
                                ps[:],
                                w_sb[:, ic, hc * 128 : (hc + 1) * 128],
                                src[:, ic, sn * 512 : (sn + 1) * 512],
                                start=(ic == 0),
                                stop=(ic == HC - 1),
                            )
                        if extra is None:
                            nc.vector.tensor_scalar_add(
                                dst[:, hc, sn * 512 : (sn + 1) * 512],
                                ps[:],
                                b_sb[:, hc : hc + 1],
                            )
                        else:
                            nc.vector.tensor_scalar(
                                dst[:, hc, sn * 512 : (sn + 1) * 512],
                                ps[:],
                                b_sb[:, hc : hc + 1],
                                extra,
                                ALU.add,
                                ALU.mult,
                            )

                hA, hB = 2 * hc, 2 * hc + 1
                for qn in range(QN):
                    pscA = pp_c.tile([VW, 512], F32, tag="pp_c")
                    pscB = pp_c.tile([VW, 512], F32, tag="pp_c")
                    for kc2 in range(SC // 2):
                        pssA = pp_s.tile([128, 1024], F32, tag="pp_s")
                        pssB = pp_s.tile([128, 1024], F32, tag="pp_s")
                        for j in range(2):
                            kc = kc2 * 2 + j
                            # A then B: bases 0 / 64 -> concurrent in array
                            for hb, pss in ((0, pssA), (64, pssB)):
                                nc.tensor.matmul(
                                    pss[:, j * 512 : (j + 1) * 512],
                                    kT[hb : hb + 64, hc, kc * 128 : (kc + 1) * 128],
                                    qT[hb : hb + 64, hc, qn * 512 : (qn + 1) * 512],
                                    start=True,
                                    stop=True,
                                )
                        etA = etpool.tile([128, 1024], F16, tag="et")
                        etB = etpool.tile([128, 1024], F16, tag="et")
                        nc.scalar.activation(etA[:], pssA[:], AF.Exp)
                        nc.scalar.activation(etB[:], pssB[:], AF.Exp)
                        for h, psc, et in ((hA, pscA, etA), (hB, pscB, etB)):
                            for j in range(2):
                                kc = kc2 * 2 + j
                                nc.tensor.matmul(
                                    psc[:],
                                    v_sb[:, kc, h * VW : (h + 1) * VW],
                                    et[:, j * 512 : (j + 1) * 512],
                                    start=(kc == 0),
                                    stop=(kc == SC - 1),
                                )
                    for h, psc in ((hA, pscA), (hB, pscB)):
                        hb = (h % 2) * 64
                        g = h * QN + qn
                        dst = ctxU[hb : hb + 64, h // 2, qn * 512 : (qn + 1) * 512]
                        if hb == 0:
                            nc.vector.tensor_copy(dst, psc[0:64, :])
                        else:
                            cst = copystage.tile([64, 512], F16, tag="cst")
                            nc.vector.tensor_copy(cst[:], psc[0:64, :])
                            nc.sync.dma_start(dst, cst[:])
                        rstage = copystage.tile([65, 512], F32, tag="rstage")
                        nc.vector.tensor_copy(rstage[64:65, :], psc[64:65, :])
                        r = grow(g)
                        nc.sync.dma_start(rows_sb[r : r + 1, :], rstage[64:65, :])

                # after half the heads are done, normalize that half so the
                # work overlaps the remaining attention chunks
                if hc == HC // 2 - 1 or hc == HC - 1:
                    first = hc == HC // 2 - 1
                    rlo = 0 if first else 32
                    rhi = rlo + NG // 2
                    glo = 0 if first else NG // 2
                    rec = copystage.tile([NR, 512], F32, tag="rec")
                    nc.vector.reciprocal(rec[rlo:rhi, :], rows_sb[rlo:rhi, :])
                    nc.vector.tensor_copy(recip16[rlo:rhi, :], rec[rlo:rhi, :])
                    for g in range(glo, glo + NG // 2):
                        normalize_group(g // QN, g % QN)

            # ---- output projection + bias + gelu ----
            out_t = out.rearrange("(n p) h -> n p h", p=128)
            for qc in range(QC):
                ost = outstage.tile([128, H], F32, tag="ost")
                for c0, cw in ((0, 512), (512, 256)):
                    ps = pp_mm.tile([128, 512], F32, tag="pp_mm")
                    for mc in range(HC):
                        nc.tensor.matmul(
                            ps[:, :cw],
                            ctxU[:, mc, qc * 128 : (qc + 1) * 128],
                            wo_sb[:, mc, c0 : c0 + cw],
                            start=(mc == 0),
                            stop=(mc == HC - 1),
                        )
                    pre = outstage.tile([128, 512], F32, tag="pre")
                    nc.vector.tensor_tensor(
                        pre[:, :cw], ps[:, :cw], bo_bc[:, c0 : c0 + cw], ALU.add
                    )
                    nc.scalar.activation(ost[:, c0 : c0 + cw], pre[:, :cw], AF.Gelu)
                nc.sync.dma_start(out_t[qc], ost[:])

    split_sync_waits(nc, cap=1)
    return nc


_NC_CACHE = None


def _get_nc():
    global _NC_CACHE
    if _NC_CACHE is None:
        _NC_CACHE = build_program()
    return _NC_CACHE


def _install_ntff_hook():
    """The image's antenv lacks axon_hooks; synthesize it so
    run_bass_kernel_spmd(trace=True) can reach the axon NTFF profiler."""
    import types

    if "antenv.axon_hooks" in sys.modules:
        return
    mod = types.ModuleType("antenv.axon_hooks")
    _h = [None]
    mod.set_axon_ntff_profile_hook = lambda h: _h.__setitem__(0, h)
    mod.get_axon_ntff_profile_hook = lambda: _h[0]
    sys.modules["antenv.axon_hooks"] = mod
    import antenv

    antenv.axon_hooks = mod
    from trn_agent_boot.trn_boot import _ntff_profile_via_ctypes

    hook = _ntff_profile_via_ctypes("/opt/axon/libaxon_pjrt.so")
    mod.set_axon_ntff_profile_hook(hook)


def kernel(
    hidden_states,
    attention_mask,
    Wq,
    bq,
    Wk,
    bk,
    Wv,
    bv,
    Wo,
    bo,
    _trace=False,
):
    from concourse.bass_utils import run_bass_kernel_spmd

    hs = np.asarray(hidden_states, dtype=np.float32)
    f16 = np.float16
    hs16 = hs.astype(f16)
    wq16 = np.asarray(Wq, dtype=np.float32).astype(f16)
    wk16 = np.asarray(Wk, dtype=np.float32).astype(f16)
    wv16 = np.asarray(Wv, dtype=np.float32).astype(f16)
    wo16 = np.asarray(Wo, dtype=np.float32).astype(f16)
    bqf = np.asarray(bq, dtype=np.float32)
    bkf = np.asarray(bk, dtype=np.float32)
    bv16v = np.asarray(bv, dtype=np.float32).astype(f16)
    bo16v = np.asarray(bo, dtype=np.float32).astype(f16)

    if _trace:
        _install_ntff_hook()
    nc = _get_nc()
    in_maps = []
    for c in range(8):
        b, qh = c // 2, c % 2
        xc = hs16[b] if qh == 0 else np.concatenate(
            [hs16[b, SQ:], hs16[b, :SQ]], axis=0
        )
        in_maps.append(
            {
                "x": xc,
                "wq": wq16,
                "wk": wk16,
                "wv": wv16,
                "wo": wo16,
                "bqf": bqf,
                "bkf": bkf,
                "bv16": bv16v,
                "bo16": bo16v,
            }
        )
    res = run_bass_kernel_spmd(
        nc, in_maps, core_ids=list(range(8)), trace=_trace
    )
    if _trace:
        kernel.last_result = res
    B = hs.shape[0]
    full = np.empty((B, S, H), dtype=np.float32)
    for c in range(8):
        b, qh = c // 2, c % 2
        full[b, qh * SQ : (qh + 1) * SQ] = res.results[c]["out"]
    return full


# revision 24
# speedup vs baseline: 1.1571x; 1.0297x over previous
"""BERT self-attention (B=4, S=2048, H=768, 12 heads) on 8 NeuronCores.

Sharding: core c handles batch b=c//2, query-half qh=c%2 (1024 q rows).
K/V are computed for the full sequence on each core (duplicated across the
2 cores of a batch) so no collectives are needed. Matmul operands are fp16
(PE runs fp16 at 1 cyc/row vs 4 for fp32; values here are O(1-40) so fp16
range is safe); accumulation stays fp32 in PSUM.

Pipeline per core (layouts chosen so the only transpose is the fp16 XBAR
DMA-transpose of the input):
  xT  [h,s]  <- DMA-transpose of x
  v   [s,h]  = xT.T @ Wv + bv, with a ones column per head (rowsum trick)
  per head-chunk hc: kT/qT chunk = W.T @ xT + b (q also *0.125), then
  attention for heads 2hc/2hc+1 interleaved so ACT exp overlaps the next
  chunk's projection matmuls. Head pair MMs alternate partition bases 0/64
  -> concurrent row-tiled execution on the PE array.
  scoresT[k,q] psum -> exp on ACT -> ctx^T accumulated via lhsT=v_aug
  (psum rows 0-63 = ctx^T, row 64 = softmax denominator).
  Normalization batched: rowsums gathered to [24,512], two reciprocals,
  per-group PE broadcast (selection matrix) + one DVE multiply in place.
  out = gelu(ctxU.T @ Wo + bo) (bias via K=1 ones matmul into psum).
"""

import sys

sys.path.insert(0, "/opt/trn_rl_repo")

import numpy as np

import concourse.bass as bass
import concourse.tile as tile
import concourse.mybir as mybir
from concourse.masks import make_identity

F16 = mybir.dt.float16
F32 = mybir.dt.float32
AF = mybir.ActivationFunctionType
ALU = mybir.AluOpType

S = 2048  # full sequence
SQ = 1024  # per-core query rows
H = 768  # hidden
NH = 12  # heads
DH = 64  # head dim
HC = H // 128  # 6 hidden chunks
SC = S // 128  # 16 seq chunks
QC = SQ // 128  # 8 query chunks
VW = DH + 1  # 65: V columns per head incl. ones column for rowsum
QN = SQ // 512  # 2 q-blocks per core
NG = NH * QN  # 24 (head, q-block) groups


def split_sync_waits(nc, cap=1):
    """Walrus in this container rejects instructions carrying more than ~1
    sync wait. Move excess waits onto same-engine NoOps inserted just before
    the instruction (same queue -> executed in order -> semantics kept)."""
    n = 0
    for b in nc.m.functions[0].blocks:
        out = []
        for inst in b.instructions:
            si = inst.sync_info
            waits = list(si.on_wait) if si is not None and si.on_wait else []
            if len(waits) > cap:
                extra, keep = waits[:-cap], waits[-cap:]
                for i in range(0, len(extra), cap):
                    nop = mybir.InstNoOp(
                        name=f"wsplit-{n}",
                        engine=inst.engine,
                        sync_info=mybir.SyncInfo(
                            on_wait=extra[i : i + cap], on_update=[]
                        ),
                    )
                    n += 1
                    out.append(nop)
                si.on_wait = keep
            out.append(inst)
        b.instructions[:] = out
    return n


def build_program():
    nc = bass.Bass()
    x = nc.declare_dram_parameter("x", [S, H], F16, isOutput=False)
    wq = nc.declare_dram_parameter("wq", [H, H], F16, isOutput=False)
    wk = nc.declare_dram_parameter("wk", [H, H], F16, isOutput=False)
    wv = nc.declare_dram_parameter("wv", [H, H], F16, isOutput=False)
    wo = nc.declare_dram_parameter("wo", [H, H], F16, isOutput=False)
    bqf = nc.declare_dram_parameter("bqf", [H], F32, isOutput=False)
    bkf = nc.declare_dram_parameter("bkf", [H], F32, isOutput=False)
    bv16 = nc.declare_dram_parameter("bv16", [H], F16, isOutput=False)
    bo16 = nc.declare_dram_parameter("bo16", [H], F16, isOutput=False)
    out = nc.declare_dram_parameter("out", [SQ, H], F32, isOutput=True)

    with tile.TileContext(nc) as tc:
        from contextlib import ExitStack

        with ExitStack() as ctx:
            consts = ctx.enter_context(tc.tile_pool(name="consts", bufs=1))
            wpool = ctx.enter_context(tc.tile_pool(name="wpool", bufs=1))
            big = ctx.enter_context(tc.tile_pool(name="big", bufs=1))
            copystage = ctx.enter_context(tc.tile_pool(name="copystage", bufs=4))
            etpool = ctx.enter_context(tc.tile_pool(name="etpool", bufs=6))
            outstage = ctx.enter_context(tc.tile_pool(name="outstage", bufs=2))
            pp_mm = ctx.enter_context(
                tc.tile_pool(name="pp_mm", bufs=2, space="PSUM")
            )

            # ---- constants ----
            ident = consts.tile([128, 128], F16, tag="ident")
            make_identity(nc, ident[:])
            ones16 = consts.tile([128, 512], F16, tag="ones16")
            nc.gpsimd.memset(ones16[:], 1.0)

            # ---- weights & biases to SBUF ----
            wq_sb = wpool.tile([128, HC, H], F16, tag="wq")
            wk_sb = wpool.tile([128, HC, H], F16, tag="wk")
            wv_sb = wpool.tile([128, HC, H], F16, tag="wv")
            wo_sb = wpool.tile([128, HC, H], F16, tag="wo")
            for w_sb, w in ((wv_sb, wv), (wk_sb, wk), (wq_sb, wq)):
                nc.scalar.dma_start(w_sb[:], w.rearrange("(c p) o -> p c o", p=128))
            nc.gpsimd.dma_start(wo_sb[:], wo.rearrange("(c p) o -> p c o", p=128))
            bq_sb = wpool.tile([128, HC], F32, tag="bq")
            bk_sb = wpool.tile([128, HC], F32, tag="bk")
            nc.gpsimd.dma_start(bq_sb[:], bqf.rearrange("(c p) -> p c", p=128))
            nc.gpsimd.dma_start(bk_sb[:], bkf.rearrange("(c p) -> p c", p=128))
            bv_sb = wpool.tile([1, H], F16, tag="bv")
            bo_sb = wpool.tile([1, H], F16, tag="bo")
            nc.gpsimd.dma_start(bv_sb[:], bv16[None, :])
            nc.gpsimd.dma_start(bo_sb[:], bo16[None, :])

            # ---- x -> xT via XBAR DMA transpose (fp16), 512-row chunks ----
            # queries are rows 0:SQ of x (host rotates the sequence so this
            # core's query half comes first; softmax over k is permutation-
            # invariant so K/V order doesn't matter). Transpose on the (idle)
            # PE: the XBAR DMA-transpose path only sustains ~60 GB/s.
            xT = big.tile([128, HC, S], F16, tag="xT")
            x_t = x.rearrange("(n p) h -> n p h", p=128)
            with tc.tile_pool(name="xstage", bufs=4) as xstage, tc.tile_pool(
                name="pp_t", bufs=2, space="PSUM"
            ) as pp_t:
                for sc in range(SC):
                    xt = xstage.tile([128, H], F16, tag="xt")
                    nc.sync.dma_start(xt[:], x_t[sc])
                    for hc in range(HC):
                        pst = pp_t.tile([128, 128], F16, tag="pp_t")
                        nc.tensor.transpose(
                            pst[:], xt[:, hc * 128 : (hc + 1) * 128], ident[:]
                        )
                        nc.vector.tensor_copy(
                            xT[:, hc, sc * 128 : (sc + 1) * 128], pst[:]
                        )

            # ---- broadcast bias rows across 128 partitions (K=1 matmul) --
            bv_bc = wpool.tile([128, H], F32, tag="bv_bc")
            bo_bc = wpool.tile([128, H], F32, tag="bo_bc")
            for bc, bsb in ((bv_bc, bv_sb), (bo_bc, bo_sb)):
                for c0, cw in ((0, 512), (512, 256)):
                    ps = pp_mm.tile([128, 512], F32, tag="pp_mm")
                    nc.tensor.matmul(
                        ps[:, :cw],
                        ones16[0:1, 0:128],
                        bsb[:, c0 : c0 + cw],
                        start=True,
                        stop=True,
                    )
                    nc.vector.tensor_copy(bc[:, c0 : c0 + cw], ps[:, :cw])

            # ---- V (seq on partitions) with ones column per head ----
            v_sb = big.tile([128, SC, NH * VW], F16, tag="v")
            v_heads = v_sb[:].rearrange("p s (h c) -> p s h c", c=VW)
            nc.gpsimd.memset(v_heads[:, :, :, DH], 1.0)
            for sc in range(SC):
                for c0, cw in ((0, 512), (512, 256)):
                    ps = pp_mm.tile([128, 512], F32, tag="pp_mm")
                    for ic in range(HC):
                        nc.tensor.matmul(
                            ps[:, :cw],
                            xT[:, ic, sc * 128 : (sc + 1) * 128],
                            wv_sb[:, ic, c0 : c0 + cw],
                            start=(ic == 0),
                            stop=(ic == HC - 1),
                        )
                    h0 = c0 // DH
                    nhh = cw // DH
                    nc.vector.scalar_tensor_tensor(
                        v_heads[:, sc, h0 : h0 + nhh, 0:DH],
                        ps[:, :cw].rearrange("p (h c) -> p h c", c=DH),
                        1.0,
                        bv_bc[:, c0 : c0 + cw].rearrange("p (h c) -> p h c", c=DH),
                        ALU.mult,
                        ALU.add,
                    )

            pp_s = ctx.enter_context(tc.tile_pool(name="pp_s", bufs=2, space="PSUM"))
            pp_c = ctx.enter_context(tc.tile_pool(name="pp_c", bufs=2, space="PSUM"))

            def emit_out_qc(qc):
                ost = outstage.tile([128, H], F32, tag="ost")
                for c0, cw in ((0, 512), (512, 256)):
                    ps = pp_mm.tile([128, 512], F32, tag="pp_mm")
                    for mc in range(HC):
                        nc.tensor.matmul(
                            ps[:, :cw],
                            ctxU[:, mc, qc * 128 : (qc + 1) * 128],
                            wo_sb[:, mc, c0 : c0 + cw],
                            start=(mc == 0),
                            stop=(mc == HC - 1),
                        )
                    pre = outstage.tile([128, 512], F32, tag="pre")
                    nc.vector.tensor_tensor(
                        pre[:, :cw], ps[:, :cw], bo_bc[:, c0 : c0 + cw], ALU.add
                    )
                    nc.scalar.activation(ost[:, c0 : c0 + cw], pre[:, :cw], AF.Gelu)
                nc.sync.dma_start(out_t[qc], ost[:])

            kT = big.tile([128, HC, S], F16, tag="kT")
            qT = big.tile([128, HC, SQ], F16, tag="qT")
            ctxU = big.tile([128, HC, SQ], F16, tag="ctxU")
            # group g lives at padded row r(g): second half starts at
            # partition 32 (compute-engine partition bases must be 0/32/64/96)
            NR = 32 + NG // 2  # 44
            rows_sb = big.tile([NR, 512], F32, tag="rows")
            recip16 = big.tile([NR, 512], F16, tag="recip16")
            nc.gpsimd.memset(recip16[:], 0.0)

            def grow(g):
                return g if g < NG // 2 else 32 + (g - NG // 2)

            def normalize_group(h, qn, pool=None, tag="pp_mm"):
                """Broadcast 1/rowsum over 64 partitions via a selection-
                matrix matmul, then scale ctxU in place."""
                hb = (h % 2) * 64
                g = h * QN + qn
                r = grow(g)
                pb = (pool or pp_mm).tile([128, 512], F32, tag=tag)
                nc.tensor.matmul(
                    pb[hb : hb + 64, :],
                    ident[0:NR, r : r + 1].to_broadcast([NR, 64]),
                    recip16[:],
                    start=True,
                    stop=True,
                )
                sl = ctxU[hb : hb + 64, h // 2, qn * 512 : (qn + 1) * 512]
                nc.vector.tensor_tensor(sl, sl, pb[hb : hb + 64, :], ALU.mult)

            # ---- per head-chunk: K/Q projection then paired attention ----
            for hc in range(HC):
                for w_sb, b_sb, dst, nsn, extra in (
                    (wk_sb, bk_sb, kT, S // 512, None),
                    (wq_sb, bq_sb, qT, QN, 0.125),
                ):
                    src = xT
                    for sn in range(nsn):
                        ps = pp_mm.tile([128, 512], F32, tag="pp_mm")
                        for ic in range(HC):
                            nc.tensor.matmul(
                                ps[:],
                                w_sb[:, ic, hc * 128 : (hc + 1) * 128],
                                src[:, ic, sn * 512 : (sn + 1) * 512],
                                start=(ic == 0),
                                stop=(ic == HC - 1),
                            )
                        if extra is None:
                            nc.vector.tensor_scalar_add(
                                dst[:, hc, sn * 512 : (sn + 1) * 512],
                                ps[:],
                                b_sb[:, hc : hc + 1],
                            )
                        else:
                            nc.vector.tensor_scalar(
                                dst[:, hc, sn * 512 : (sn + 1) * 512],
                                ps[:],
                                b_sb[:, hc : hc + 1],
                                extra,
                                ALU.add,
                                ALU.mult,
                            )

                if hc >= HC // 2:
                    # normalize 4 first-half groups per remaining chunk
                    base = (hc - HC // 2) * 4
                    for g in range(base, base + 4):
                        normalize_group(g // QN, g % QN)

                hA, hB = 2 * hc, 2 * hc + 1
                for qn in range(QN):
                    pscA = pp_c.tile([VW, 512], F32, tag="pp_c")
                    pscB = pp_c.tile([VW, 512], F32, tag="pp_c")
                    for kc2 in range(SC // 2):
                        pssA = pp_s.tile([128, 1024], F32, tag="pp_s")
                        pssB = pp_s.tile([128, 1024], F32, tag="pp_s")
                        for j in range(2):
                            kc = kc2 * 2 + j
                            # A then B: bases 0 / 64 -> concurrent in array
                            for hb, pss in ((0, pssA), (64, pssB)):
                                nc.tensor.matmul(
                                    pss[:, j * 512 : (j + 1) * 512],
                                    kT[hb : hb + 64, hc, kc * 128 : (kc + 1) * 128],
                                    qT[hb : hb + 64, hc, qn * 512 : (qn + 1) * 512],
                                    start=True,
                                    stop=True,
                                )
                        etA = etpool.tile([128, 1024], F16, tag="et")
                        etB = etpool.tile([128, 1024], F16, tag="et")
                        nc.scalar.activation(etA[:], pssA[:], AF.Exp)
                        nc.scalar.activation(etB[:], pssB[:], AF.Exp)
                        for h, psc, et in ((hA, pscA, etA), (hB, pscB, etB)):
                            for j in range(2):
                                kc = kc2 * 2 + j
                                nc.tensor.matmul(
                                    psc[:],
                                    v_sb[:, kc, h * VW : (h + 1) * VW],
                                    et[:, j * 512 : (j + 1) * 512],
                                    start=(kc == 0),
                                    stop=(kc == SC - 1),
                                )
                    for h, psc in ((hA, pscA), (hB, pscB)):
                        hb = (h % 2) * 64
                        g = h * QN + qn
                        dst = ctxU[hb : hb + 64, h // 2, qn * 512 : (qn + 1) * 512]
                        if hb == 0:
                            nc.vector.tensor_copy(dst, psc[0:64, :])
                        else:
                            cst = copystage.tile([64, 512], F16, tag="cst")
                            nc.vector.tensor_copy(cst[:], psc[0:64, :])
                            nc.sync.dma_start(dst, cst[:])
                        rstage = copystage.tile([65, 512], F32, tag="rstage")
                        nc.vector.tensor_copy(rstage[64:65, :], psc[64:65, :])
                        r = grow(g)
                        nc.sync.dma_start(rows_sb[r : r + 1, :], rstage[64:65, :])

                # reciprocal for the first half of the heads as soon as
                # they are done; their normalize matmuls are spread across
                # the remaining head-chunks (emitted at the top of the loop)
                if hc == HC // 2 - 1:
                    rec = copystage.tile([NR, 512], F32, tag="rec")
                    nc.vector.reciprocal(rec[0 : NG // 2, :], rows_sb[0 : NG // 2, :])
                    nc.vector.tensor_copy(
                        recip16[0 : NG // 2, :], rec[0 : NG // 2, :]
                    )

            # ---- tail: second-half recip, normalize by q-block, project --
            rec2 = copystage.tile([NR, 512], F32, tag="rec")
            nc.vector.reciprocal(rec2[32:NR, :], rows_sb[32:NR, :])
            nc.vector.tensor_copy(recip16[32:NR, :], rec2[32:NR, :])
            out_t = out.rearrange("(n p) h -> n p h", p=128)
            for qn in range(QN):
                for h in range(NH // 2, NH):
                    normalize_group(h, qn)
                for qc in range(qn * QC // 2, (qn + 1) * QC // 2):
                    emit_out_qc(qc)

    split_sync_waits(nc, cap=1)
    return nc


_NC_CACHE = None


def _get_nc():
    global _NC_CACHE
    if _NC_CACHE is None:
        _NC_CACHE = build_program()
    return _NC_CACHE


def _install_ntff_hook():
    """The image's antenv lacks axon_hooks; synthesize it so
    run_bass_kernel_spmd(trace=True) can reach the axon NTFF profiler."""
    import types

    if "antenv.axon_hooks" in sys.modules:
        return
    mod = types.ModuleType("antenv.axon_hooks")
    _h = [None]
    mod.set_axon_ntff_profile_hook = lambda h: _h.__setitem__(0, h)
    mod.get_axon_ntff_profile_hook = lambda: _h[0]
    sys.modules["antenv.axon_hooks"] = mod
    import antenv

    antenv.axon_hooks = mod
    from trn_agent_boot.trn_boot import _ntff_profile_via_ctypes

    hook = _ntff_profile_via_ctypes("/opt/axon/libaxon_pjrt.so")
    mod.set_axon_ntff_profile_hook(hook)


def kernel(
    hidden_states,
    attention_mask,
    Wq,
    bq,
    Wk,
    bk,
    Wv,
    bv,
    Wo,
    bo,
    _trace=False,
):
    from concourse.bass_utils import run_bass_kernel_spmd

    hs = np.asarray(hidden_states, dtype=np.float32)
    f16 = np.float16
    hs16 = hs.astype(f16)
    wq16 = np.asarray(Wq, dtype=np.float32).astype(f16)
    wk16 = np.asarray(Wk, dtype=np.float32).astype(f16)
    wv16 = np.asarray(Wv, dtype=np.float32).astype(f16)
    wo16 = np.asarray(Wo, dtype=np.float32).astype(f16)
    bqf = np.asarray(bq, dtype=np.float32)
    bkf = np.asarray(bk, dtype=np.float32)
    bv16v = np.asarray(bv, dtype=np.float32).astype(f16)
    bo16v = np.asarray(bo, dtype=np.float32).astype(f16)

    if _trace:
        _install_ntff_hook()
    nc = _get_nc()
    in_maps = []
    for c in range(8):
        b, qh = c // 2, c % 2
        xc = hs16[b] if qh == 0 else np.concatenate(
            [hs16[b, SQ:], hs16[b, :SQ]], axis=0
        )
        in_maps.append(
            {
                "x": xc,
                "wq": wq16,
                "wk": wk16,
                "wv": wv16,
                "wo": wo16,
                "bqf": bqf,
                "bkf": bkf,
                "bv16": bv16v,
                "bo16": bo16v,
            }
        )
    res = run_bass_kernel_spmd(
        nc, in_maps, core_ids=list(range(8)), trace=_trace
    )
    if _trace:
        kernel.last_result = res
    B = hs.shape[0]
    full = np.empty((B, S, H), dtype=np.float32)
    for c in range(8):
        b, qh = c // 2, c % 2
        full[b, qh * SQ : (qh + 1) * SQ] = res.results[c]["out"]
    return full
